# revision 1
# baseline (speedup 1.0000x reference)
"""Trainium2 Bass kernel for BotanHadamardTransform: y = x @ H, with
x [4, 4096, 4096] f32 and H [4096, 4096] f32 the normalized Sylvester
Hadamard matrix H_4096 / 64.

Algorithm: Sylvester Hadamard matrices factor as Kronecker products,
H_4096 = H_A (x) H_B with A*B = 4096. For a row vector v (len 4096),
v @ H_4096 = FWHT_A applied across the A axis of (v.reshape(A, B) @ H_B).
This reduces per-row work from O(n^2) to O(n*(B + log2 A)).

Mapping to hardware (per core, 1/8 of the 16384 rows = 2048 rows):
  - host pre-transposes x so the device sees xT [4096, 2048] with the
    contraction dim on partitions (natural matmul layout, no on-device
    transposes)
  - PE contracts the low B=512 of each k-index against Hf = H[0:512,0:512]
    (which equals H_512/64 exactly) as fp32r matmuls, N=512 moving columns
  - the high A=8 factor is a 3-stage FWHT butterfly across 128-partition
    chunks; stage 1 runs fused with PSUM eviction (ScalarE evicts one
    accumulator, VectorE adds/subs against the other still in PSUM);
    stages 2-3 are whole-block VectorE ops with fully contiguous access
    patterns, with an optional thin GpSimd chunk slice
  - output is written transposed (yT [4096, 2048]); host transposes back

Buffer scheme per r-tile (R=512 moving columns, 4 r-tiles per core):
  G1 blocks (xinb, f32 [128,8,512]): DMA-in dest; dead after rounding;
     reused as stage-1 output (the butterfly ping); s2 reads them.
  xr blocks (f32r): rounded matmul input; dead after matmuls; slots
     reused for stage-2 outputs (f32 bitcast view); s3 reads those.
  s3 writes fresh G1-pool blocks; DMA-out drains them.
"""
import os
import sys

sys.path.insert(0, "/opt/trn_rl_repo")

import numpy as np

import concourse.bass as bass  # noqa: F401
import concourse.tile as tile
from concourse import bacc, mybir
from concourse.bass_utils import run_bass_kernel_spmd

N_CORES = 8
N = 4096            # hidden dim
ROWS = 4 * 4096     # total rows
RC = ROWS // N_CORES  # rows (columns of xT) per core = 2048

B = 512             # PE-contracted Kronecker factor (Hf = H_512/64)
R = 512             # moving columns per r-tile

A = N // B               # butterfly factor (8)
SUB = B // 128           # accumulating matmuls per output chunk (4)
NCH = N // 128           # 32 chunks of 128 partitions
BCH = 2 * SUB            # chunks per pair-block (8)
NPAIR = A // 2           # pair blocks (4)
QH = 2                   # q-values per PSUM half-block


def _build():
    nc = bacc.Bacc("TRN2", target_bir_lowering=False, debug=False,
                   num_devices=N_CORES)
    xT_ap = nc.dram_tensor("xT", [N, RC], mybir.dt.float32,
                           kind="ExternalInput").ap()
    hf_ap = nc.dram_tensor("Hf", [B, B], mybir.dt.float32,
                           kind="ExternalInput").ap()
    yT_ap = nc.dram_tensor("yT", [N, RC], mybir.dt.float32,
                           kind="ExternalOutput").ap()

    f32 = mybir.dt.float32
    f32r = mybir.dt.float32r

    xT_v = xT_ap.rearrange("(c p) r -> p c r", p=128)   # [128, NCH, RC]
    yT_v = yT_ap.rearrange("(c p) r -> p c r", p=128)

    n_rt = RC // R

    with tile.TileContext(nc) as tc:
        with (
            tc.tile_pool(name="hfp", bufs=1) as hfp,
            tc.tile_pool(name="xbin", bufs=2) as xbinp,
            tc.tile_pool(name="xr", bufs=2) as xrp,
            tc.tile_pool(name="g13", bufs=5) as g13p,
            tc.tile_pool(name="g2", bufs=3) as g2p,
            tc.tile_pool(name="ev", bufs=1) as evp,
            tc.tile_pool(name="ps", bufs=2, space="PSUM") as psp,
        ):
            # stationary Hf: stage f32 via an xr-pool slot, round to f32r.
            # layout: hf[p, s*B + col] = Hf[s*128 + p, col]
            hf_st = xrp.tile([128, SUB * B], f32, tag="xr", name="hf_stage")
            for s in range(SUB):
                nc.sync.dma_start(hf_st[:, s * B:(s + 1) * B],
                                  hf_ap[s * 128:(s + 1) * 128, :])
            hf_mm = hfp.tile([128, SUB * B], f32r, tag="hfr")
            nc.scalar.copy(hf_mm[:], hf_st[:])

            def hf_block(s, q):
                # lhsT block [k=128 (i2 sub s), m=128 (j2 sub q)]
                return hf_mm[:, s * B + q * 128: s * B + q * 128 + 128]

            def bf_pair(dst_add, dst_sub, src0, src1, gp_ch=2):
                """dst_add = src0+src1, dst_sub = src0-src1 on [128,BCH,R]
                tiles. GpSimd takes the trailing gp_ch chunks of each op
                (measured costs: DVE ~0.8us + 0.7us/chunk per op, GpSimd
                ~3.5us + 1.0us/chunk -> 2 chunks balances the two engines
                at ~10us per pair), VectorE the rest; both run in parallel
                with fully contiguous access patterns."""
                c_gp = BCH - gp_ch
                for (eng, c0, c1) in (("v", 0, c_gp), ("g", c_gp, BCH)):
                    if c0 >= c1:
                        continue
                    sl = lambda t: t[:, c0:c1, :].rearrange("p c r -> p (c r)")
                    if eng == "v":
                        nc.vector.tensor_add(sl(dst_add), sl(src0), sl(src1))
                        nc.vector.tensor_sub(sl(dst_sub), sl(src0), sl(src1))
                    else:
                        nc.gpsimd.tensor_add(sl(dst_add), sl(src0), sl(src1))
                        nc.gpsimd.tensor_sub(sl(dst_sub), sl(src0), sl(src1))

            for it in range(n_rt):
                r0 = it * R
                g1 = []   # stage-1 output tiles
                for m in range(NPAIR):
                    ch0 = m * BCH
                    xb = xbinp.tile([128, BCH, R], f32, tag="xbin",
                                    name=f"xb_{it}_{m}")
                    g1m = g13p.tile([128, BCH, R], f32, tag="g13",
                                    name=f"g1_{it}_{m}")
                    g1.append(g1m)
                    nc.sync.dma_start(xb[:],
                                      xT_v[:, ch0:ch0 + BCH, r0:r0 + R])
                    # rounding pass f32 -> f32r (ScalarE); xb is dead after
                    # this and becomes the stage-1 destination
                    xg = xrp.tile([128, BCH, R], f32r, tag="xr",
                                  name=f"xg_{it}_{m}")
                    nc.scalar.copy(xg[:], xb[:])

                    for qh in range(SUB // QH):
                        pg = [psp.tile([128, QH * R], f32, tag=f"pg{j}",
                                       name=f"pg{j}_{it}_{m}_{qh}")
                              for j in range(2)]
                        for qq in range(QH):
                            q = qh * QH + qq
                            for s in range(SUB):
                                for j in range(2):
                                    nc.tensor.matmul(
                                        pg[j][:, qq * R:(qq + 1) * R],
                                        hf_block(s, q),
                                        xg[:, j * SUB + s, :],
                                        start=(s == 0),
                                        stop=(s == SUB - 1),
                                    )
                        # stage-1 butterfly fused with eviction: ScalarE
                        # evicts pg0 to a scratch tile, VectorE adds/subs
                        # against pg1 still in PSUM (DVE has one PSUM port)
                        ev = evp.tile([128, QH * R], f32, tag="ev",
                                      name=f"ev_{it}_{m}_{qh}")
                        nc.scalar.copy(ev[:], pg[0][:])
                        ca = qh * QH
                        cb = SUB + qh * QH
                        oa = g1m[:, ca:ca + QH, :].rearrange(
                            "p c r -> p (c r)")
                        ob = g1m[:, cb:cb + QH, :].rearrange(
                            "p c r -> p (c r)")
                        nc.vector.tensor_add(oa, ev[:], pg[1][:])
                        nc.vector.tensor_sub(ob, ev[:], pg[1][:])

                # remaining stages: block-pair adds; xr slots freed by the
                # matmuls become the f32 destinations via fresh pool tiles
                if A == 4:
                    g2 = [g2p.tile([128, BCH, R], f32, tag="g2",
                                   name=f"g2_{it}_{i}") for i in range(2)]
                    bf_pair(g2[0], g2[1], g1[0], g1[1], 2)
                    for i in range(2):
                        nc.scalar.dma_start(
                            yT_v[:, i * BCH:(i + 1) * BCH, r0:r0 + R],
                            g2[i][:])
                else:  # A == 8
                    g2 = [g2p.tile([128, BCH, R], f32, tag="g2",
                                   name=f"g2_{it}_{i}") for i in range(4)]
                    bf_pair(g2[0], g2[1], g1[0], g1[1], 2)
                    bf_pair(g2[2], g2[3], g1[2], g1[3], 2)

                    # stage 3: outputs in final chunk order
                    g3 = [g13p.tile([128, BCH, R], f32, tag="g13",
                                   name=f"g3_{it}_{i}") for i in range(4)]
                    bf_pair(g3[0], g3[2], g2[0], g2[2], 2)
                    bf_pair(g3[1], g3[3], g2[1], g2[3], 2)

                    for i in range(4):
                        nc.scalar.dma_start(
                            yT_v[:, i * BCH:(i + 1) * BCH, r0:r0 + R],
                            g3[i][:])

    nc.compile()
    return nc


_prog = None


def _get_prog():
    global _prog
    if _prog is None:
        _prog = _build()
    return _prog


def _run(xT, Hf, trace=False):
    nc = _get_prog()
    in_maps = [
        {"xT": np.ascontiguousarray(xT[:, c * RC:(c + 1) * RC]), "Hf": Hf}
        for c in range(N_CORES)
    ]
    res = run_bass_kernel_spmd(nc, in_maps, core_ids=list(range(N_CORES)),
                               trace=trace)
    return res


def kernel(x, H):
    x = np.asarray(x)
    H = np.asarray(H)
    xT = np.ascontiguousarray(x.reshape(ROWS, N).T)          # [N, ROWS]
    Hf = np.ascontiguousarray(H[:B, :B])                      # = H_B / 64
    res = _run(xT, Hf)
    y = np.empty((ROWS, N), dtype=np.float32)
    for c in range(N_CORES):
        y[c * RC:(c + 1) * RC, :] = res.results[c]["yT"].T
    return y.reshape(4, 4096, N)



# revision 6
# speedup vs baseline: 1.0401x; 1.0401x over previous
"""Trainium2 Bass kernel for BotanHadamardTransform: y = x @ H, with
x [4, 4096, 4096] f32 and H [4096, 4096] f32 the normalized Sylvester
Hadamard matrix H_4096 / 64.

Algorithm: Sylvester Hadamard matrices factor as Kronecker products,
H_4096 = H_A (x) H_B with A*B = 4096.  For a row vector v (len 4096)
viewed as blocks v[A, B]:  v @ H_4096 = (FWHT_A over the A axis) then
(per-block @ H_B).  The two factors commute, so the log2(A)-stage
radix-2 FWHT butterfly runs on the INPUT, entirely in bf16 (2 elem/
cycle on DVE), and the PE contracts only the B factor.

Everything on device is bf16 (matmul accumulation stays fp32 in PSUM):
the host downcasts x and H (H entries are +-2^-6, exact in bf16) and
upcasts the result; tolerance for this problem is 2e-2 and the bf16
pipeline measures ~5e-3.  bf16 also halves HBM traffic and makes
weight loads use the fast path (f32r LDWEIGHTS was ~40% of PE time in
the f32r version of this kernel).

Mapping per core (1/8 of the 16384 rows = 2048 rows = columns of xT):
  - host pre-transposes x so the device sees xT [4096, 2048] bf16 with
    the contraction dim on partitions; output is written transposed
    (yT [4096, 2048] bf16) and the host transposes/upcasts back
  - per r-tile (R=512 columns) and r-half (256): DMA-in
    [128, 32 chunks, 256], run the log2(A) butterfly stages as big
    chunk-slab add/subs, each op split along r between DVE (bf16 2x
    mode) and GpSimd; the final stage writes the matmul moving tile
  - PE: per output group of 4 chunks, SUB accumulating bf16 matmuls
    per chunk against stationary 128x128 blocks of Hb = H[:B,:B]
  - ScalarE evicts each 4-chunk PSUM group straight to bf16; output
    DMAs ride the Activation HWDGE ring, input DMAs the Sync ring
"""
import os
import sys

sys.path.insert(0, "/opt/trn_rl_repo")

import numpy as np
import ml_dtypes

import concourse.bass as bass  # noqa: F401
import concourse.tile as tile
from concourse import bacc, mybir
from concourse.bass_utils import run_bass_kernel_spmd

NP_BF16 = np.dtype(ml_dtypes.bfloat16)

N_CORES = 8
N = 4096            # hidden dim
ROWS = 4 * 4096     # total rows
RC = ROWS // N_CORES  # rows (columns of xT) per core = 2048

A = 16              # FWHT butterfly factor
B = N // A          # PE-contracted Kronecker factor (Hb = H_B / 64)
SUB = B // 128      # accumulating matmuls / chunks per A-block
NCH = N // 128      # 32 chunks of 128 partitions

R = 512             # moving columns per r-tile
RH = 256            # r-half granularity for DMA-in + butterfly
DR = 160            # butterfly r-split: DVE gets [0:DR], GpSimd [DR:RH]


def _build():
    nc = bacc.Bacc("TRN2", target_bir_lowering=False, debug=False,
                   num_devices=N_CORES)
    bf16 = mybir.dt.bfloat16
    f32 = mybir.dt.float32

    xT_ap = nc.dram_tensor("xT", [N, RC], bf16, kind="ExternalInput").ap()
    hb_ap = nc.dram_tensor("Hb", [B, B], bf16, kind="ExternalInput").ap()
    yT_ap = nc.dram_tensor("yT", [N, RC], bf16, kind="ExternalOutput").ap()

    xT_v = xT_ap.rearrange("(c p) r -> p c r", p=128)   # [128, NCH, RC]
    yT_v = yT_ap.rearrange("(c p) r -> p c r", p=128)

    n_rt = RC // R
    n_half = R // RH
    n_stage = A.bit_length() - 1   # log2(A)

    with tile.TileContext(nc) as tc:
        with (
            tc.tile_pool(name="hbp", bufs=1) as hbp,
            tc.tile_pool(name="pin", bufs=3) as pinp,
            tc.tile_pool(name="pa", bufs=2) as pap,
            tc.tile_pool(name="pb", bufs=2) as pbp,
            tc.tile_pool(name="pmv", bufs=2) as pmvp,
            tc.tile_pool(name="pev", bufs=4) as pevp,
            tc.tile_pool(name="ps", bufs=2, space="PSUM") as psp,
        ):
            # stationary Hb, loaded directly in bf16:
            # hb[p, s*B + col] = Hb[s*128 + p, col]
            hb = hbp.tile([128, SUB * B], bf16, tag="hb")
            for s in range(SUB):
                nc.sync.dma_start(hb[:, s * B:(s + 1) * B],
                                  hb_ap[s * 128:(s + 1) * 128, :])

            def hb_block(s, q):
                # lhsT block [k=128 (i2 sub s), m=128 (j2 sub q)]
                return hb[:, s * B + q * 128: s * B + q * 128 + 128]

            def bf_op(dst, src0, src1, sub):
                """dst = src0 -+ src1 on pre-sliced [128, c, RH] APs,
                r-split between DVE and GpSimd (both all-bf16; DVE runs
                the packed-2-byte fast mode)."""
                for (eng, r0, r1) in (("v", 0, DR), ("g", DR, RH)):
                    d = dst[:, :, r0:r1]
                    a = src0[:, :, r0:r1]
                    b = src1[:, :, r0:r1]
                    e = nc.vector if eng == "v" else nc.gpsimd
                    if sub:
                        e.tensor_sub(d, a, b)
                    else:
                        e.tensor_add(d, a, b)

            for it in range(n_rt):
                r0 = it * R
                # matmul moving tile for this r-tile (last stage writes it)
                mv = pmvp.tile([128, NCH, R], bf16, tag="mv",
                               name=f"mv_{it}")
                for ih in range(n_half):
                    h0 = ih * RH
                    xin = pinp.tile([128, NCH, RH], bf16, tag="pin",
                                    name=f"xin_{it}_{ih}")
                    nc.sync.dma_start(
                        xin[:], xT_v[:, :, r0 + h0:r0 + h0 + RH])

                    # log2(A) radix-2 FWHT stages over the A axis
                    # (block = SUB chunks); ping-pong pa/pb, last stage
                    # lands in the r-half slice of mv.
                    src = xin
                    for st in range(n_stage):
                        d = (A >> 1) >> st          # block distance
                        w = d * SUB                  # op width in chunks
                        last = st == n_stage - 1
                        if last:
                            dst = mv[:, :, h0:h0 + RH]
                        else:
                            pool, ptag = ((pap, "pa") if st % 2 == 0
                                          else (pbp, "pb"))
                            dst = pool.tile([128, NCH, RH], bf16,
                                            tag=ptag,
                                            name=f"t{st}_{it}_{ih}")
                        for g in range(0, NCH, 2 * w):
                            lo = src[:, g:g + w, :]
                            hi = src[:, g + w:g + 2 * w, :]
                            bf_op(dst[:, g:g + w, :], lo, hi, sub=False)
                            bf_op(dst[:, g + w:g + 2 * w, :], lo, hi,
                                  sub=True)
                        src = dst

                # PE contraction + eviction, per output group of 4 chunks
                for gg in range(NCH // 4):
                    pp = psp.tile([128, 4, R], f32, tag="ps",
                                  name=f"pp_{it}_{gg}")
                    j1s = sorted({(4 * gg + i) // SUB for i in range(4)})
                    for q in range(SUB):
                        for s in range(SUB):
                            for ji, j1 in enumerate(j1s):
                                nc.tensor.matmul(
                                    pp[:, ji * SUB + q, :],
                                    hb_block(s, q),
                                    mv[:, j1 * SUB + s, :],
                                    start=(s == 0),
                                    stop=(s == SUB - 1),
                                )
                    ev = pevp.tile([128, 4, R], bf16, tag="pev",
                                   name=f"ev_{it}_{gg}")
                    nc.scalar.copy(ev[:].rearrange("p c r -> p (c r)"),
                                   pp[:].rearrange("p c r -> p (c r)"))
                    nc.scalar.dma_start(
                        yT_v[:, 4 * gg:4 * gg + 4, r0:r0 + R], ev[:])

    nc.compile()
    return nc


_prog = None


def _get_prog():
    global _prog
    if _prog is None:
        _prog = _build()
    return _prog


def _run(xT, Hb, trace=False):
    nc = _get_prog()
    in_maps = [
        {"xT": np.ascontiguousarray(xT[:, c * RC:(c + 1) * RC]), "Hb": Hb}
        for c in range(N_CORES)
    ]
    res = run_bass_kernel_spmd(nc, in_maps, core_ids=list(range(N_CORES)),
                               trace=trace)
    return res


def kernel(x, H):
    x = np.asarray(x)
    H = np.asarray(H)
    xT = np.ascontiguousarray(
        x.reshape(ROWS, N).T.astype(NP_BF16))             # [N, ROWS] bf16
    Hb = np.ascontiguousarray(H[:B, :B].astype(NP_BF16))  # = H_B / 64
    res = _run(xT, Hb)
    y = np.empty((ROWS, N), dtype=np.float32)
    for c in range(N_CORES):
        y[c * RC:(c + 1) * RC, :] = res.results[c]["yT"].T.astype(np.float32)
    return y.reshape(4, 4096, N)


# revision 9
# speedup vs baseline: 1.2649x; 1.2162x over previous
"""Trainium2 Bass kernel for BotanHadamardTransform: y = x @ H, with
x [4, 4096, 4096] f32 and H [4096, 4096] f32 the normalized Sylvester
Hadamard matrix H_4096 / 64.

Algorithm: Sylvester Hadamard matrices factor as Kronecker products,
H_4096 = H_A (x) H_B with A*B = 4096.  For a row vector v (len 4096)
viewed as blocks v[A, B]:  v @ H_4096 = (FWHT_A over the A axis) then
(per-block @ H_B).  The two factors commute, so the log2(A)-stage
radix-2 FWHT butterfly runs on the INPUT, entirely in bf16 (2 elem/
cycle on DVE), and the PE contracts only the B factor.

Everything on device is bf16 (matmul accumulation stays fp32 in PSUM):
the host downcasts x and H (H entries are +-2^-6, exact in bf16) and
upcasts the result; tolerance for this problem is 2e-2 and the bf16
pipeline measures ~5e-3.  bf16 also halves HBM traffic and makes
weight loads use the fast path (f32r LDWEIGHTS was ~40% of PE time in
the f32r version of this kernel).

Mapping per core (1/8 of the 16384 rows = 2048 rows = columns of xT):
  - host pre-transposes x so the device sees xT [4096, 2048] bf16 with
    the contraction dim on partitions; output is written transposed
    (yT [4096, 2048] bf16) and the host transposes/upcasts back
  - per r-tile (R=512 columns) and r-half (256): DMA-in
    [128, 32 chunks, 256], run the log2(A) butterfly stages as big
    chunk-slab add/subs, each op split along r between DVE (bf16 2x
    mode) and GpSimd; the final stage writes the matmul moving tile
  - PE: per output group of 4 chunks, SUB accumulating bf16 matmuls
    per chunk against stationary 128x128 blocks of Hb = H[:B,:B]
  - ScalarE evicts each 4-chunk PSUM group straight to bf16; output
    DMAs ride the Activation HWDGE ring, input DMAs the Sync ring
"""
import os
import sys

sys.path.insert(0, "/opt/trn_rl_repo")

import numpy as np
import ml_dtypes

import concourse.bass as bass  # noqa: F401
import concourse.tile as tile
from concourse import bacc, mybir
from concourse.bass_utils import run_bass_kernel_spmd

NP_BF16 = np.dtype(ml_dtypes.bfloat16)

N_CORES = 8
N = 4096            # hidden dim
ROWS = 4 * 4096     # total rows
RC = ROWS // N_CORES  # rows (columns of xT) per core = 2048

A = 16              # FWHT butterfly factor
B = N // A          # PE-contracted Kronecker factor (Hb = H_B / 64)
SUB = B // 128      # accumulating matmuls / chunks per A-block
NCH = N // 128      # 32 chunks of 128 partitions

R = 512             # moving columns per r-tile
RH = 256            # r-half granularity for DMA-in + butterfly
DR = 200            # butterfly r-split: DVE gets [0:DR], GpSimd [DR:RH]
                    # (measured rates: DVE bf16 ~0.55 ns/elem with the 2x
                    # packed mode, GpSimd ~1.92 ns/elem -> 78/22 split)


def _build():
    nc = bacc.Bacc("TRN2", target_bir_lowering=False, debug=False,
                   num_devices=N_CORES)
    bf16 = mybir.dt.bfloat16
    f32 = mybir.dt.float32

    xT_ap = nc.dram_tensor("xT", [N, RC], bf16, kind="ExternalInput").ap()
    hb_ap = nc.dram_tensor("Hb", [B, B], bf16, kind="ExternalInput").ap()
    yT_ap = nc.dram_tensor("yT", [N, RC], bf16, kind="ExternalOutput").ap()

    xT_v = xT_ap.rearrange("(c p) r -> p c r", p=128)   # [128, NCH, RC]
    yT_v = yT_ap.rearrange("(c p) r -> p c r", p=128)

    n_rt = RC // R
    n_half = R // RH
    n_stage = A.bit_length() - 1   # log2(A)

    with tile.TileContext(nc) as tc:
        with (
            tc.tile_pool(name="hbp", bufs=1) as hbp,
            tc.tile_pool(name="pin", bufs=3) as pinp,
            tc.tile_pool(name="pa", bufs=2) as pap,
            tc.tile_pool(name="pb", bufs=2) as pbp,
            tc.tile_pool(name="pmv", bufs=2) as pmvp,
            tc.tile_pool(name="pev", bufs=4) as pevp,
            tc.tile_pool(name="ps", bufs=2, space="PSUM") as psp,
        ):
            # stationary Hb, loaded directly in bf16:
            # hb[p, s*B + col] = Hb[s*128 + p, col]
            hb = hbp.tile([128, SUB * B], bf16, tag="hb")
            for s in range(SUB):
                nc.sync.dma_start(hb[:, s * B:(s + 1) * B],
                                  hb_ap[s * 128:(s + 1) * 128, :])

            def hb_block(s, q):
                # lhsT block [k=128 (i2 sub s), m=128 (j2 sub q)]
                return hb[:, s * B + q * 128: s * B + q * 128 + 128]

            def bf_stage(dst, src, w):
                """One radix-2 FWHT stage of slab width w chunks on
                [128, NCH, RH] tiles: pairs of consecutive w-chunk slabs
                (lo, hi) -> (lo+hi, lo-hi), as ONE 4D-strided op per
                (engine lane, add/sub).  r-split between DVE (fast 2x
                packed bf16 mode) and GpSimd."""
                dv = dst.rearrange("p (g w) r -> p g w r", w=w)
                sv = src.rearrange("p (g w) r -> p g w r", w=w)
                for (e, r0, r1) in ((nc.vector, 0, DR),
                                    (nc.gpsimd, DR, RH)):
                    lo = sv[:, 0::2, :, r0:r1]
                    hi = sv[:, 1::2, :, r0:r1]
                    e.tensor_add(dv[:, 0::2, :, r0:r1], lo, hi)
                    e.tensor_sub(dv[:, 1::2, :, r0:r1], lo, hi)

            for it in range(n_rt):
                r0 = it * R
                # matmul moving tile for this r-tile (last stage writes it)
                mv = pmvp.tile([128, NCH, R], bf16, tag="mv",
                               name=f"mv_{it}")
                for ih in range(n_half):
                    h0 = ih * RH
                    xin = pinp.tile([128, NCH, RH], bf16, tag="pin",
                                    name=f"xin_{it}_{ih}")
                    nc.sync.dma_start(
                        xin[:], xT_v[:, :, r0 + h0:r0 + h0 + RH])

                    # log2(A) radix-2 FWHT stages over the A axis
                    # (block = SUB chunks); ping-pong pa/pb, last stage
                    # lands in the r-half slice of mv.
                    src = xin
                    for st in range(n_stage):
                        d = (A >> 1) >> st          # block distance
                        w = d * SUB                  # slab width in chunks
                        last = st == n_stage - 1
                        if last:
                            dst = mv[:, :, h0:h0 + RH]
                        else:
                            pool, ptag = ((pap, "pa") if st % 2 == 0
                                          else (pbp, "pb"))
                            dst = pool.tile([128, NCH, RH], bf16,
                                            tag=ptag,
                                            name=f"t{st}_{it}_{ih}")
                        bf_stage(dst, src, w)
                        src = dst

                # PE contraction + eviction, per output group of 4 chunks
                for gg in range(NCH // 4):
                    pp = psp.tile([128, 4, R], f32, tag="ps",
                                  name=f"pp_{it}_{gg}")
                    j1s = sorted({(4 * gg + i) // SUB for i in range(4)})
                    for q in range(SUB):
                        for s in range(SUB):
                            for ji, j1 in enumerate(j1s):
                                nc.tensor.matmul(
                                    pp[:, ji * SUB + q, :],
                                    hb_block(s, q),
                                    mv[:, j1 * SUB + s, :],
                                    start=(s == 0),
                                    stop=(s == SUB - 1),
                                )
                    ev = pevp.tile([128, 4, R], bf16, tag="pev",
                                   name=f"ev_{it}_{gg}")
                    nc.scalar.copy(ev[:].rearrange("p c r -> p (c r)"),
                                   pp[:].rearrange("p c r -> p (c r)"))
                    nc.scalar.dma_start(
                        yT_v[:, 4 * gg:4 * gg + 4, r0:r0 + R], ev[:])

    nc.compile()
    return nc


_prog = None


def _get_prog():
    global _prog
    if _prog is None:
        _prog = _build()
    return _prog


def _run(xT, Hb, trace=False):
    nc = _get_prog()
    in_maps = [
        {"xT": np.ascontiguousarray(xT[:, c * RC:(c + 1) * RC]), "Hb": Hb}
        for c in range(N_CORES)
    ]
    res = run_bass_kernel_spmd(nc, in_maps, core_ids=list(range(N_CORES)),
                               trace=trace)
    return res


def kernel(x, H):
    x = np.asarray(x)
    H = np.asarray(H)
    xT = np.ascontiguousarray(
        x.reshape(ROWS, N).T.astype(NP_BF16))             # [N, ROWS] bf16
    Hb = np.ascontiguousarray(H[:B, :B].astype(NP_BF16))  # = H_B / 64
    res = _run(xT, Hb)
    y = np.empty((ROWS, N), dtype=np.float32)
    for c in range(N_CORES):
        y[c * RC:(c + 1) * RC, :] = res.results[c]["yT"].T.astype(np.float32)
    return y.reshape(4, 4096, N)


# revision 10
# speedup vs baseline: 2.0317x; 1.6062x over previous
"""Trainium2 Bass kernel for BotanHadamardTransform: y = x @ H, with
x [4, 4096, 4096] f32 and H = H_4096/64 the normalized Sylvester
Hadamard matrix.

Factorization: H_4096 = H_32 (x) H_128.  For row-blocks v[32, 128]:
y = (FWHT_32 over the block axis) then (per-block @ H_128/64).  The
five radix-2 FWHT stages commute; we run distances 8, 4, 2 on DVE
(bf16, 2x packed mode) and FOLD distances 16 and 1 into the PE as a
4-term accumulation with +-H_128 stationaries:

  y[c]  (c = base + o0 + 16*o4, base in {0,2,..,14})
      = H^T sum_{s0,s4} (-1)^(o0 s0 + o4 s4) u3[base + s0 + 16 s4]

Why this shape (all from measurement on this part):
  - GpSimd tensor ops run ~1.92 ns/elem and CONTEND with DVE's 2x mode
    for the shared SBUF port pair: running both is slower than DVE
    alone (0.55 ns/elem).  So the butterfly is DVE-only, 3 stages.
  - LDWEIGHTS costs ~97 ns and is not hidden; with only two stationary
    matrices (+H_128, -H_128) the PE does 16-matmul bursts per LDW
    pair.  Per 2-group burst: 32 mm + 2 LDW.
  - Everything bf16 on the wire (host casts; tol 2e-2 vs ~5e-3 here):
    halves DMA (the ~106us floor) and doubles DVE rate.

Layout per core (1/8 of rows): xT [4096, 2048] bf16 in, yT out.
r-tiles of R=256 columns; DMA-in per 16-chunk half so the butterfly
starts after 1 MB.  PSUM groups = m-pair -> 8 output chunks as two
4-chunk runs; ScalarE evicts to bf16, out-DMAs alternate DMA rings.
"""
import os
import sys

sys.path.insert(0, "/opt/trn_rl_repo")

import numpy as np
import ml_dtypes

import concourse.bass as bass  # noqa: F401
import concourse.tile as tile
from concourse import bacc, mybir
from concourse.bass_utils import run_bass_kernel_spmd

NP_BF16 = np.dtype(ml_dtypes.bfloat16)

N_CORES = 8
N = 4096            # hidden dim
ROWS = 4 * 4096     # total rows
RC = ROWS // N_CORES  # rows (columns of xT) per core = 2048

B = 128             # PE-contracted factor (Hb = H_128 / 64)
NCH = N // 128      # 32 chunks of 128 partitions (chunk == A-block)
R = 512             # moving columns per r-tile (matmul n=512)
HCH = 16            # chunks per DMA/butterfly half


def _build():
    nc = bacc.Bacc("TRN2", target_bir_lowering=False, debug=False,
                   num_devices=N_CORES)
    bf16 = mybir.dt.bfloat16
    f32 = mybir.dt.float32

    xT_ap = nc.dram_tensor("xT", [N, RC], bf16, kind="ExternalInput").ap()
    # [H_128/64 | -H_128/64], horizontally packed
    hb_ap = nc.dram_tensor("Hb2", [B, 2 * B], bf16,
                           kind="ExternalInput").ap()
    yT_ap = nc.dram_tensor("yT", [N, RC], bf16, kind="ExternalOutput").ap()

    xT_v = xT_ap.rearrange("(c p) r -> p c r", p=128)   # [128, NCH, RC]
    yT_v = yT_ap.rearrange("(c p) r -> p c r", p=128)

    n_rt = RC // R

    # fold sign structure: output o=(o4,o0), term t=(s4,s0);
    # sign = (-1)^(o0*s0 + o4*s4)
    OUTS = [(0, 0), (0, 1), (1, 0), (1, 1)]
    TERMS = [(0, 0), (0, 1), (1, 0), (1, 1)]
    PLUS = [(o, t) for o in OUTS for t in TERMS
            if (o[1] * t[1] + o[0] * t[0]) % 2 == 0]
    MINUS = [(o, t) for o in OUTS for t in TERMS
             if (o[1] * t[1] + o[0] * t[0]) % 2 == 1]

    with tile.TileContext(nc) as tc:
        with (
            tc.tile_pool(name="hbp", bufs=1) as hbp,
            tc.tile_pool(name="pin", bufs=3) as pinp,
            tc.tile_pool(name="pa", bufs=2) as pap,
            tc.tile_pool(name="pb", bufs=2) as pbp,
            tc.tile_pool(name="pmv", bufs=2) as pmvp,
            tc.tile_pool(name="pev", bufs=4) as pevp,
            tc.tile_pool(name="ps", bufs=2, space="PSUM") as psp,
        ):
            hb = hbp.tile([128, 2 * B], bf16, tag="hb")
            nc.sync.dma_start(hb[:], hb_ap[:, :])
            Hp = hb[:, 0:B]        # +H_128/64
            Hm = hb[:, B:2 * B]    # -H_128/64

            def stage4d(dst, src, w, nch):
                """Radix-2 stage, slab width w chunks over nch chunks:
                one DVE 4D op per add/sub (no GpSimd: shared-port)."""
                dv = dst.rearrange("p (g w) r -> p g w r", w=w)
                sv = src.rearrange("p (g w) r -> p g w r", w=w)
                lo = sv[:, 0::2, :, :]
                hi = sv[:, 1::2, :, :]
                nc.vector.tensor_add(dv[:, 0::2, :, :], lo, hi)
                nc.vector.tensor_sub(dv[:, 1::2, :, :], lo, hi)

            for it in range(n_rt):
                r0 = it * R
                mv = pmvp.tile([128, NCH, R], bf16, tag="mv",
                               name=f"mv_{it}")
                pbs = []
                for ih in range(2):      # 16-chunk halves
                    c0 = ih * HCH
                    xin = pinp.tile([128, HCH, R], bf16, tag="pin",
                                    name=f"xin_{it}_{ih}")
                    nc.sync.dma_start(
                        xin[:], xT_v[:, c0:c0 + HCH, r0:r0 + R])
                    # d8 within the half (pairs (c, c+8))
                    t1 = pap.tile([128, HCH, R], bf16, tag="pa",
                                  name=f"t1_{it}_{ih}")
                    stage4d(t1, xin, 8, HCH)
                    # d4 within the half
                    t2 = pbp.tile([128, HCH, R], bf16, tag="pb",
                                  name=f"t2_{it}_{ih}")
                    stage4d(t2, t1, 4, HCH)
                    # d2 within the half -> mv
                    stage4d(mv[:, c0:c0 + HCH, :], t2, 2, HCH)

                # PE: fold d16 and d1; per-m groups (outputs 2m, 2m+1,
                # 2m+16, 2m+17), +H burst (10 mm) then -H (6 mm)
                order = [("p", o, t) for (o, t) in PLUS] + \
                        [("m", o, t) for (o, t) in MINUS]
                first = {}
                last = {}
                for i, (ph, o, t) in enumerate(order):
                    if o not in first:
                        first[o] = i
                    last[o] = i
                for m in range(8):
                    pp = psp.tile([128, 4, R], f32, tag="ps",
                                  name=f"pp_{it}_{m}")
                    for i, (ph, o, t) in enumerate(order):
                        o4, o0 = o
                        s4, s0 = t
                        st = Hp if ph == "p" else Hm
                        nc.tensor.matmul(
                            pp[:, o4 * 2 + o0, :],
                            st,
                            mv[:, 2 * m + s0 + 16 * s4, :],
                            start=(first[o] == i),
                            stop=(last[o] == i),
                        )
                    ev = pevp.tile([128, 4, R], bf16, tag="pev",
                                   name=f"ev_{it}_{m}")
                    nc.scalar.copy(ev[:].rearrange("p c r -> p (c r)"),
                                   pp[:].rearrange("p c r -> p (c r)"))
                    eng = nc.sync if m % 2 == 0 else nc.scalar
                    eng.dma_start(
                        yT_v[:, 2 * m:2 * m + 2, r0:r0 + R],
                        ev[:, 0:2, :])
                    eng.dma_start(
                        yT_v[:, 2 * m + 16:2 * m + 18, r0:r0 + R],
                        ev[:, 2:4, :])

    nc.compile()
    return nc


_prog = None


def _get_prog():
    global _prog
    if _prog is None:
        _prog = _build()
    return _prog


def _run(xT, Hb2, trace=False):
    nc = _get_prog()
    in_maps = [
        {"xT": np.ascontiguousarray(xT[:, c * RC:(c + 1) * RC]),
         "Hb2": Hb2}
        for c in range(N_CORES)
    ]
    res = run_bass_kernel_spmd(nc, in_maps, core_ids=list(range(N_CORES)),
                               trace=trace)
    return res


def _make_hb2(H):
    Hb = np.asarray(H)[:B, :B]
    return np.ascontiguousarray(
        np.concatenate([Hb, -Hb], axis=1).astype(NP_BF16))


def kernel(x, H):
    x = np.asarray(x)
    H = np.asarray(H)
    xT = np.ascontiguousarray(
        x.reshape(ROWS, N).T.astype(NP_BF16))             # [N, ROWS] bf16
    Hb2 = _make_hb2(H)
    res = _run(xT, Hb2)
    y = np.empty((ROWS, N), dtype=np.float32)
    for c in range(N_CORES):
        y[c * RC:(c + 1) * RC, :] = res.results[c]["yT"].T.astype(np.float32)
    return y.reshape(4, 4096, N)


# revision 14
# speedup vs baseline: 2.0400x; 1.0041x over previous
"""Trainium2 Bass kernel for BotanHadamardTransform: y = x @ H, with
x [4, 4096, 4096] f32 and H = H_4096/64 the normalized Sylvester
Hadamard matrix.

Factorization: H_4096 = H_32 (x) H_128.  For row-blocks v[32, 128]:
y = (FWHT_32 over the block axis) then (per-block @ H_128/64).  The
five radix-2 FWHT stages commute; we run distances 8, 4, 2 on DVE
(bf16, 2x packed mode) and FOLD distances 16 and 1 into the PE as a
4-term accumulation with +-H_128 stationaries:

  y[c]  (c = base + o0 + 16*o4, base in {0,2,..,14})
      = H^T sum_{s0,s4} (-1)^(o0 s0 + o4 s4) u3[base + s0 + 16 s4]

Why this shape (all from measurement on this part):
  - GpSimd tensor ops run ~1.92 ns/elem and CONTEND with DVE's 2x mode
    for the shared SBUF port pair: running both is slower than DVE
    alone (0.55 ns/elem).  So the butterfly is DVE-only, 3 stages.
  - LDWEIGHTS costs ~97 ns and is not hidden; with only two stationary
    matrices (+H_128, -H_128) the PE does 16-matmul bursts per LDW
    pair.  Per 2-group burst: 32 mm + 2 LDW.
  - Everything bf16 on the wire (host casts; tol 2e-2 vs ~5e-3 here):
    halves DMA (the ~106us floor) and doubles DVE rate.

Layout per core (1/8 of rows): xT [4096, 2048] bf16 in, yT out.
r-tiles of R=256 columns; DMA-in per 16-chunk half so the butterfly
starts after 1 MB.  PSUM groups = m-pair -> 8 output chunks as two
4-chunk runs; ScalarE evicts to bf16, out-DMAs alternate DMA rings.
"""
import os
import sys

sys.path.insert(0, "/opt/trn_rl_repo")

import numpy as np
import ml_dtypes

import concourse.bass as bass  # noqa: F401
import concourse.tile as tile
from concourse import bacc, mybir
from concourse.bass_utils import run_bass_kernel_spmd

NP_BF16 = np.dtype(ml_dtypes.bfloat16)

N_CORES = 8
N = 4096            # hidden dim
ROWS = 4 * 4096     # total rows
RC = ROWS // N_CORES  # rows (columns of xT) per core = 2048

B = 128             # PE-contracted factor (Hb = H_128 / 64)
NCH = N // 128      # 32 chunks of 128 partitions (chunk == A-block)
R = 512             # moving columns per r-tile (matmul n=512)
HCH = 16            # chunks per DMA/butterfly half


def _build():
    nc = bacc.Bacc("TRN2", target_bir_lowering=False, debug=False,
                   num_devices=N_CORES)
    bf16 = mybir.dt.bfloat16
    f32 = mybir.dt.float32

    xT_ap = nc.dram_tensor("xT", [N, RC], bf16, kind="ExternalInput").ap()
    # [H_128/64 | -H_128/64], horizontally packed
    hb_ap = nc.dram_tensor("Hb2", [B, 2 * B], bf16,
                           kind="ExternalInput").ap()
    yT_ap = nc.dram_tensor("yT", [N, RC], bf16, kind="ExternalOutput").ap()

    xT_v = xT_ap.rearrange("(c p) r -> p c r", p=128)   # [128, NCH, RC]
    yT_v = yT_ap.rearrange("(c p) r -> p c r", p=128)

    n_rt = RC // R

    # fold sign structure: output o=(o4,o0), term t=(s4,s0);
    # sign = (-1)^(o0*s0 + o4*s4)
    OUTS = [(0, 0), (0, 1), (1, 0), (1, 1)]
    TERMS = [(0, 0), (0, 1), (1, 0), (1, 1)]
    PLUS = [(o, t) for o in OUTS for t in TERMS
            if (o[1] * t[1] + o[0] * t[0]) % 2 == 0]
    MINUS = [(o, t) for o in OUTS for t in TERMS
             if (o[1] * t[1] + o[0] * t[0]) % 2 == 1]

    with tile.TileContext(nc) as tc:
        with (
            tc.tile_pool(name="hbp", bufs=1) as hbp,
            tc.tile_pool(name="pin", bufs=3) as pinp,
            tc.tile_pool(name="pa", bufs=2) as pap,
            tc.tile_pool(name="pb", bufs=2) as pbp,
            tc.tile_pool(name="pmv", bufs=2) as pmvp,
            tc.tile_pool(name="pev", bufs=4) as pevp,
            tc.tile_pool(name="ps", bufs=2, space="PSUM") as psp,
        ):
            hb = hbp.tile([128, 2 * B], bf16, tag="hb")
            nc.sync.dma_start(hb[:], hb_ap[:, :])
            Hp = hb[:, 0:B]        # +H_128/64
            Hm = hb[:, B:2 * B]    # -H_128/64

            def stage4d(dst, src, w, nch):
                """Radix-2 stage, slab width w chunks over nch chunks:
                one DVE 4D op per add/sub (no GpSimd: shared-port)."""
                dv = dst.rearrange("p (g w) r -> p g w r", w=w)
                sv = src.rearrange("p (g w) r -> p g w r", w=w)
                lo = sv[:, 0::2, :, :]
                hi = sv[:, 1::2, :, :]
                nc.vector.tensor_add(dv[:, 0::2, :, :], lo, hi)
                nc.vector.tensor_sub(dv[:, 1::2, :, :], lo, hi)

            for it in range(n_rt):
                r0 = it * R
                mv = pmvp.tile([128, NCH, R], bf16, tag="mv",
                               name=f"mv_{it}")
                pbs = []
                for ih in range(2):      # 16-chunk halves
                    c0 = ih * HCH
                    xin = pinp.tile([128, HCH, R], bf16, tag="pin",
                                    name=f"xin_{it}_{ih}")
                    nc.sync.dma_start(
                        xin[:], xT_v[:, c0:c0 + HCH, r0:r0 + R])
                    # d8 within the half (pairs (c, c+8))
                    t1 = pap.tile([128, HCH, R], bf16, tag="pa",
                                  name=f"t1_{it}_{ih}")
                    stage4d(t1, xin, 8, HCH)
                    # d4 within the half
                    t2 = pbp.tile([128, HCH, R], bf16, tag="pb",
                                  name=f"t2_{it}_{ih}")
                    stage4d(t2, t1, 4, HCH)
                    # d2 within the half -> mv
                    stage4d(mv[:, c0:c0 + HCH, :], t2, 2, HCH)

                # PE: fold d16 and d1; per-m groups (outputs 2m, 2m+1,
                # 2m+16, 2m+17), +H burst (10 mm) then -H (6 mm)
                order = [("p", o, t) for (o, t) in PLUS] + \
                        [("m", o, t) for (o, t) in MINUS]
                first = {}
                last = {}
                for i, (ph, o, t) in enumerate(order):
                    if o not in first:
                        first[o] = i
                    last[o] = i
                for m in range(8):
                    pp = psp.tile([128, 4, R], f32, tag="ps",
                                  name=f"pp_{it}_{m}")
                    for i, (ph, o, t) in enumerate(order):
                        o4, o0 = o
                        s4, s0 = t
                        st = Hp if ph == "p" else Hm
                        nc.tensor.matmul(
                            pp[:, o4 * 2 + o0, :],
                            st,
                            mv[:, 2 * m + s0 + 16 * s4, :],
                            start=(first[o] == i),
                            stop=(last[o] == i),
                        )
                    ev = pevp.tile([128, 4, R], bf16, tag="pev",
                                   name=f"ev_{it}_{m}")
                    nc.scalar.copy(ev[:].rearrange("p c r -> p (c r)"),
                                   pp[:].rearrange("p c r -> p (c r)"))
                    eng = nc.sync if m % 2 == 0 else nc.scalar
                    eng.dma_start(
                        yT_v[:, 2 * m:2 * m + 2, r0:r0 + R],
                        ev[:, 0:2, :])
                    eng.dma_start(
                        yT_v[:, 2 * m + 16:2 * m + 18, r0:r0 + R],
                        ev[:, 2:4, :])

    nc.compile()
    return nc


_prog = None


def _get_prog():
    global _prog
    if _prog is None:
        _prog = _build()
    return _prog


def _run(xT, Hb2, trace=False):
    nc = _get_prog()
    in_maps = [
        {"xT": np.ascontiguousarray(xT[:, c * RC:(c + 1) * RC]),
         "Hb2": Hb2}
        for c in range(N_CORES)
    ]
    res = run_bass_kernel_spmd(nc, in_maps, core_ids=list(range(N_CORES)),
                               trace=trace)
    return res


def _make_hb2(H):
    Hb = np.asarray(H)[:B, :B]
    return np.ascontiguousarray(
        np.concatenate([Hb, -Hb], axis=1).astype(NP_BF16))


def kernel(x, H):
    x = np.asarray(x)
    H = np.asarray(H)
    xT = np.ascontiguousarray(
        x.reshape(ROWS, N).T.astype(NP_BF16))             # [N, ROWS] bf16
    Hb2 = _make_hb2(H)
    res = _run(xT, Hb2)
    y = np.empty((ROWS, N), dtype=np.float32)
    for c in range(N_CORES):
        y[c * RC:(c + 1) * RC, :] = res.results[c]["yT"].T.astype(np.float32)
    return y.reshape(4, 4096, N)
